# revision 53
# baseline (speedup 1.0000x reference)
"""Trainium2 Bass kernel for nn_BottomLevelDecoderRNN (fp8 DoubleRow version).

Structure exploited: the recurrent state is reset at every bar boundary
(t % 16 == 0) and `notes` is teacher-forced from `target`, so the 16 bars
of 16 steps each are fully independent. We run a 16-step loop with
(bar, batch) vmapped into a 256-wide column dimension per core (batch
sharded 8 ways; 16 bars x 16 batch = 256 columns).

Datapath (per LSTM cell eval):
- All recurrent/weight matmuls are fp8e4m3 with MatmulPerfMode.DoubleRow
  (2 k-tiles folded per instruction, 0.5 cycles/row): weights are scaled
  x16 into fp8's normal range; the activation un-scales via its free
  `scale` operand. The precomputed per-column additive gate input (token
  embedding term + c_t term + biases, also x16) is injected into PSUM via
  an fp16 identity matmul.
- Gates live in two PSUM tiles per cell: [128,6,256] (i,f,o -> one batched
  Sigmoid) + [128,2,256] (g -> Tanh, emitted first so its bank frees
  early). Pointwise tail on DVE in fp16; h is kept in fp16 (feeds the
  fp16 out-projection so output error stays fp16-grade) and converted
  to an fp8 copy on the idle GPSIMD engine for the next matmul rhs.
- Output projection stays fully fp16 (fp8 there would put ~3% directly
  on the logits). bout is added on the host (zero in this problem).

Layouts: feature dim folded [128 partitions x 2 chunks]; h/c/activation
tiles are [128, 2, 256] = [partition, fold chunk, (bar,batch) column].
Weights [128, m, t, 128] = [k-in-chunk partition, out chunk, k-tile,
out-in-chunk]; gate order after PERM4H is [i, f, o, g].
"""

import numpy as np
import ml_dtypes

import concourse.bacc as bacc
import concourse.mybir as mybir
import concourse.tile as tile
from concourse.bass import MemorySpace
from concourse.bass_utils import run_bass_kernel_spmd
from concourse.masks import make_identity

B, Dd, Hh, Vv = 128, 256, 256, 130
NB = 16          # bars
BL = B // 8      # batch per core
R = NB * BL      # columns per core = 256
S = 16           # steps per bar
NCORES = 8
F8 = mybir.dt.float8e4
F16 = mybir.dt.float16
F32 = mybir.dt.float32
AF = mybir.ActivationFunctionType
DR = mybir.MatmulPerfMode.DoubleRow
WS = 16.0        # weight/psum scale
NPF8 = ml_dtypes.float8_e4m3

last_result = None  # BassKernelResults of the most recent run (for profiling)
_prog_cache = {}
_dbg_labels = {}   # instruction name -> emission-site label (for analysis)
_cur_label = [""]

CFG = dict(
    hh_resid=False,   # residual-compensated fp8 for the W1_hh recurrence
    pair1=False,      # share tanh(c2) between again2 and lstm2_0
    pair2=False,      # share tanh(c2) between lstm2_1 and next-step vmap0
    rat_l=0,          # lstm2 cells whose tanh(c2) moves to a DVE rational
)

# gate-block permutation of the 4H dim: [i, f, o, g]
PERM4H = np.r_[0:256, 256:512, 768:1024, 512:768]


def _w8(W16):
    """W16 [4H, K] (perm'd, pre-scaled) -> fp8 [128, 8, K/128, 128] DR
    weight layout: out[p, m, t, u] = W16[m*128+u, t*128+p]."""
    G, K = W16.shape
    assert G == 1024
    t = K // 128
    arr = W16.reshape(8, 128, t, 128).transpose(3, 0, 2, 1)
    return np.ascontiguousarray(arr).astype(NPF8)


def _x16(x):
    """x [R, 4H] fp32 -> f16 [128, 8, 256] gate-dim-folded layout, x16."""
    arr = (WS * x).T.reshape(8, 128, R).transpose(1, 0, 2)
    return np.ascontiguousarray(arr).astype(np.float16)


def _fold(M):
    """M [R, K] -> [128, K/128, R]: tile[p, k, r] = M[r, k*128+p]."""
    Rr, K = M.shape
    arr = M.T.reshape(K // 128, 128, Rr).transpose(1, 0, 2)
    return np.ascontiguousarray(arr)


def _build_program(key):
    use_ctx_bias, cfg = key
    cfg = dict(cfg)
    hh_resid = cfg["hh_resid"]
    use_pair1 = cfg["pair1"]
    use_pair2 = cfg["pair2"]
    rat_l = cfg["rat_l"]
    nc = bacc.Bacc(None, target_bir_lowering=False)

    def _label_cb(ins):
        _dbg_labels[ins.name] = _cur_label[0]
    nc._state.push_inst_callback(_label_cb)

    mm = nc.tensor.matmul

    def L(x):
        _cur_label[0] = x

    # ---- DRAM I/O ----
    d_w1h = nc.dram_tensor("w1h", [3, 128, 8, 2, 128], F8, kind="ExternalInput")
    d_w1n = nc.dram_tensor("w1n", [3, 128, 8, 2, 128], F8, kind="ExternalInput")
    d_wc = nc.dram_tensor("wc", [128, 8, 8, 128], F8, kind="ExternalInput")
    if hh_resid:
        d_w1hr = nc.dram_tensor("w1hr", [3, 128, 8, 2, 128], F8,
                                kind="ExternalInput")
    d_wo = nc.dram_tensor("wo", [3, 128, 2, 130], F16, kind="ExternalInput")
    d_xc1 = nc.dram_tensor("xc1", [3, 128, 8, 256], F16, kind="ExternalInput")
    d_hinit8 = nc.dram_tensor("hinit8", [128, 2, 256], F8, kind="ExternalInput")
    d_hinit16 = nc.dram_tensor("hinit16", [128, 2, 256], F16,
                               kind="ExternalInput")
    d_xa0 = nc.dram_tensor("xa0", [3, 128, 8, 256], F16, kind="ExternalInput")
    d_xb = nc.dram_tensor("xb", [S, 3, 128, 8, 256], F16, kind="ExternalInput")
    if use_ctx_bias:
        d_bcb = nc.dram_tensor("bcb", [128, 8, 256], F16, kind="ExternalInput")
    d_out = nc.dram_tensor("out", [S, 3, 130, R], F16, kind="ExternalOutput")

    from contextlib import ExitStack
    with tile.TileContext(nc) as tc, ExitStack() as es:
        const = es.enter_context(tc.tile_pool(name="const", bufs=1))
        psum = es.enter_context(tc.tile_pool(name="psum", bufs=2,
                                             space=MemorySpace.PSUM))
        act = es.enter_context(tc.tile_pool(name="act", bufs=3))
        scr = es.enter_context(tc.tile_pool(name="scr", bufs=3))
        stg = es.enter_context(tc.tile_pool(name="stg", bufs=3))
        npool = es.enter_context(tc.tile_pool(name="npool", bufs=3))
        hpool = es.enter_context(tc.tile_pool(name="hpool", bufs=4))
        cpool = es.enter_context(tc.tile_pool(name="cpool", bufs=2))

        def cload(name, dram_ap, shape, dtype):
            t = const.tile(shape, dtype, tag=name)
            nc.sync.dma_start(t[:], dram_ap)
            return t

        # consts needed first load first
        hinit8 = cload("hinit8", d_hinit8[:], [128, 2, 256], F8)
        hinit16 = cload("hinit16", d_hinit16[:], [128, 2, 256], F16)
        xa0, w1h = [], []
        for i in range(3):
            xa0.append(cload(f"xa0_{i}", d_xa0[i], [128, 8, 256], F16))
            w1h.append(cload(f"w1h_{i}", d_w1h[i], [128, 8, 2, 128], F8))
        wc = cload("wc", d_wc[:], [128, 8, 8, 128], F8)
        w1n = [cload(f"w1n_{i}", d_w1n[i], [128, 8, 2, 128], F8)
               for i in range(3)]
        w1hr = [cload(f"w1hr_{i}", d_w1hr[i], [128, 8, 2, 128], F8)
                for i in range(3)] if hh_resid else None
        xc1 = [cload(f"xc1_{i}", d_xc1[i], [128, 8, 256], F16)
               for i in range(3)]
        wo = [cload(f"wo_{i}", d_wo[i], [128, 2, 130], F16) for i in range(3)]
        bcb = cload("bcb", d_bcb[:], [128, 8, 256], F16) if use_ctx_bias \
            else None

        SPLIT_TANH = bool(__import__('os').environ.get('SPLIT_TANH'))
        PRIO = 3000  # chain-critical ops outrank same-step fillers
        ident = const.tile([128, 128], F16, tag="ident")
        make_identity(nc, ident[:])
        zeros = const.tile([128, 2, 256], F16, tag="zeros")
        nc.gpsimd.memset(zeros[:], 0.0)

        # ---- state (python-side handles; tiles rotate via pools) ----
        st_h8 = {}
        st_h16 = {}
        st_c = {}
        for nm in ["h1_0", "h1_1", "h1_2", "h2_0", "h2_1", "h2_2", "hc"]:
            st_h8[nm] = hinit8
            st_h16[nm] = hinit16
            st_c[nm] = zeros[:]

        def cell_matmuls(wpairs, xadd, tag, defer=0):
            """Emit gate matmuls for one cell. wpairs: list of (w_tile,
            rhs_h8_tile, ktile_base); each contributes one DoubleRow
            matmul (2 k-tiles) per out chunk. xadd: [128,8,256] f16 tile
            or None (injected per PSUM bank via fp16 identity matmul).
            The last `defer` weight pairs (the latest-ready rhs, e.g. hc
            in the ctx chain) are returned as a finisher so other cells'
            matmuls can be emitted into the PE stream before them.
            Returns (g_psum[128,2,256], ifo_psum[128,6,256], finish_fn)."""
            gt = psum.tile([128, 2, 256], F32, tag="g", name=f"{tag}_g")
            it = psum.tile([128, 6, 256], F32, tag="ifo", name=f"{tag}_ifo")
            banks = [(gt, 0, 6), (it, 0, 0), (it, 2, 2), (it, 4, 4)]
            started = {}

            def emit(pairs, last_group):
                for wi, (wt, rhs, kb) in enumerate(pairs):
                    lastw = last_group and wi == len(pairs) - 1
                    for bi, (th, j0, moff) in enumerate(banks):
                        for j in range(2):
                            mm(th[:, j0 + j, :],
                               wt[:, moff + j, kb:kb + 2, :], rhs[:, 0:2, :],
                               start=not started.get(bi, False),
                               stop=lastw and j == 1, perf_mode=DR,
                               skip_group_check=True)
                            started[bi] = True

            if xadd is not None:
                for bi, (th, j0, moff) in enumerate(banks):
                    mm(th[:, j0:j0 + 2, :], ident[:],
                       xadd[:, moff:moff + 2, :],
                       start=True, stop=False, skip_group_check=True)
                    started[bi] = True
            head = wpairs[:len(wpairs) - defer] if defer else wpairs
            tail = wpairs[len(wpairs) - defer:] if defer else []
            emit(head, last_group=not tail)
            if not tail:
                return gt, it, None
            return gt, it, (lambda: emit(tail, last_group=True))

        def tail_pre(gt, it, cname, c_dst):
            """Gate activations + c update (written to c_dst AP); returns
            the sigmoid tile for tail_fin."""
            tg = act.tile([128, 2, 256], F16, tag="tg")
            nc.scalar.activation(tg[:], gt[:], AF.Tanh, scale=1.0 / WS)
            a1 = act.tile([128, 6, 256], F16, tag="a1")
            nc.scalar.activation(a1[:], it[:], AF.Sigmoid, scale=1.0 / WS)
            m1 = scr.tile([128, 2, 256], F16, tag="m1")
            nc.vector.tensor_mul(m1[:], a1[:, 0:2, :], tg[:])
            t1 = scr.tile([128, 2, 256], F16, tag="t1")
            nc.vector.tensor_mul(t1[:], a1[:, 2:4, :], st_c[cname])
            nc.vector.tensor_add(c_dst, t1[:], m1[:])
            st_c[cname] = c_dst
            return a1

        def tail_fin(a1, tc2, cname, mode):
            """h output from sigmoid(o) x tanh(c2).
            mode: 'h8only'  (again/ctx: h16 never consumed; fp8 out on DVE,
                             shortest latency to the next matmul rhs)
                  'h8first' (vmap: fp8 out first to unblock ctx0, then the
                             fp16 copy for the out-projection)
                  'h16pool' (lstm2: h8 only needed next step; convert on
                             the idle GPSIMD, off the critical path)."""
            so = a1[:, 4:6, :]
            if mode == "h8only":
                h8 = hpool.tile([128, 2, 256], F8, tag=f"h8_{cname}")
                nc.vector.tensor_mul(h8[:], so, tc2)
                st_h8[cname] = h8
                return
            if mode == "h8first":
                h8 = hpool.tile([128, 2, 256], F8, tag=f"h8_{cname}")
                nc.vector.tensor_mul(h8[:], so, tc2)
                h16 = hpool.tile([128, 2, 256], F16, tag=f"h16_{cname}")
                nc.vector.tensor_mul(h16[:], so, tc2)
                st_h8[cname] = h8
                st_h16[cname] = h16
                return
            h16 = hpool.tile([128, 2, 256], F16, tag=f"h16_{cname}")
            nc.vector.tensor_mul(h16[:], so, tc2)
            h8 = hpool.tile([128, 2, 256], F8, tag=f"h8_{cname}")
            nc.gpsimd.tensor_add(h8[:], h16[:], zeros[:])
            st_h16[cname] = h16
            st_h8[cname] = h8

        def dve_tanh(dst, x):
            """tanh via clamped Pade(5,4) on the DVE: frees the Act
            engine (the bottleneck) for latency-tolerant cells.
            max abs error ~1e-3."""
            A = mybir.AluOpType
            t = scr.tile([128, 2, 256], F16, tag="rt_t")
            nc.vector.tensor_scalar(t[:], x, 3.5, -3.5, A.min, A.max)
            x2 = scr.tile([128, 2, 256], F16, tag="rt_x2")
            nc.vector.tensor_mul(x2[:], t[:], t[:])
            u = scr.tile([128, 2, 256], F16, tag="rt_u")
            nc.vector.tensor_scalar(u[:], x2[:], 105.0, None, A.add)
            v = scr.tile([128, 2, 256], F16, tag="rt_v")
            nc.vector.tensor_mul(v[:], u[:], x2[:])
            w = scr.tile([128, 2, 256], F16, tag="rt_w")
            nc.vector.tensor_scalar(w[:], v[:], 1.0 / 15.0, 63.0,
                                    A.mult, A.add)
            num = scr.tile([128, 2, 256], F16, tag="rt_n")
            nc.vector.tensor_mul(num[:], w[:], t[:])
            d1 = scr.tile([128, 2, 256], F16, tag="rt_d1")
            nc.vector.tensor_scalar(d1[:], x2[:], 25.53255, None, A.add)
            d2 = scr.tile([128, 2, 256], F16, tag="rt_d2")
            nc.vector.tensor_scalar(d2[:], x2[:], 2.46745, None, A.add)
            den = scr.tile([128, 2, 256], F16, tag="rt_de")
            nc.vector.tensor_mul(den[:], d1[:], d2[:])
            r = scr.tile([128, 2, 256], F16, tag="rt_r")
            with nc.allow_low_precision(reason="den in [63,1100], fp16 ok"):
                nc.vector.reciprocal(r[:], den[:])
            nc.vector.tensor_mul(dst, num[:], r[:])

        def cell_tail(gt, it, cname, mode="h8first", rat=False,
                      ctx_style=False):
            c_new = cpool.tile([128, 2, 256], F16, tag=f"c_{cname}")
            a1 = tail_pre(gt, it, cname, c_new[:])
            tc2 = scr.tile([128, 2, 256], F16, tag="tc2")
            if rat:
                dve_tanh(tc2[:], c_new[:])
            else:
                nc.scalar.activation(tc2[:], c_new[:], AF.Tanh)
            tail_fin(a1, tc2[:], cname, mode)

        def pair_tail(nameA, modeA, nameB, modeB):
            """Two filler cells share one [128,4,256] c tile so their
            tanh(c2) runs as a single Act instruction. pre(0)/pre(1) emit
            at each cell's position; fin() after both."""
            cp = cpool.tile([128, 4, 256], F16, tag=f"cp_{nameA}_{nameB}")
            a1s = {}

            def pre(which, gt, it):
                nm = nameA if which == 0 else nameB
                a1s[which] = tail_pre(gt, it, nm,
                                      cp[:, 2 * which:2 * which + 2, :])

            def fin():
                tcp = scr.tile([128, 4, 256], F16, tag="tcp")
                if SPLIT_TANH:
                    nc.scalar.activation(tcp[:, 0:2, :], cp[:, 0:2, :], AF.Tanh)
                    nc.scalar.activation(tcp[:, 2:4, :], cp[:, 2:4, :], AF.Tanh)
                else:
                    nc.scalar.activation(tcp[:], cp[:], AF.Tanh)
                tail_fin(a1s[0], tcp[:, 0:2, :], nameA, modeA)
                tail_fin(a1s[1], tcp[:, 2:4, :], nameB, modeB)
            return pre, fin

        def lstm1_mm(i, xadd, tag):
            pairs = [(w1h[i], st_h8[f"h1_{i}"], 0)]
            if w1hr is not None:
                pairs.append((w1hr[i], st_h8[f"h1_{i}"], 0))
            g, it_, fin = cell_matmuls(pairs, xadd, tag)
            return g, it_

        def ctx_mm(it, rhs, tag, defer=0):
            """rhs: {h1_0, h1_1, h1_2, hc} -> h8 tiles for THIS iteration
            (reference semantics: h1_j is post-again for j < it, post-vmap
            otherwise). Pairs ordered oldest-ready first so the freshest
            operand streams last."""
            order = [["hc", "h1_0", "h1_1", "h1_2"],
                     ["h1_1", "h1_2", "hc", "h1_0"],
                     ["h1_2", "h1_0", "hc", "h1_1"]][it]
            kb = {"h1_0": 0, "h1_1": 2, "h1_2": 4, "hc": 6}
            pairs = [(wc, rhs[nm], kb[nm]) for nm in order]
            return cell_matmuls(pairs, bcb, tag, defer=defer)

        def lstm2_mm(i, tag, defer=0):
            pairs = [(w1h[i], st_h8[f"h2_{i}"], 0)]
            if w1hr is not None:
                pairs.append((w1hr[i], st_h8[f"h2_{i}"], 0))
            pairs.append((w1n[i], st_h8["hc"], 0))
            return cell_matmuls(pairs, xc1[i], tag, defer=defer)

        def out_proj(i, h1v16, s):
            # logits = Wout @ h1v + Wout @ h2, accumulated in PSUM (no
            # pre-added hsum: keeps the GPSIMD off the critical path).
            tout = psum.tile([128, 2, 256], F32, tag="g", name="tout")
            rhs2 = [h1v16, st_h16[f"h2_{i}"]]
            srcs = [(h, k) for h in rhs2 for k in range(2)]
            for n, (h, k) in enumerate(srcs):
                mm(tout[:, 0, :], wo[i][:, k, 0:128], h[:, k, :],
                   start=(n == 0), stop=(n == 3), skip_group_check=True)
            for n, (h, k) in enumerate(srcs):
                mm(tout[0:2, 1, :], wo[i][:, k, 128:130], h[:, k, :],
                   start=(n == 0), stop=(n == 3), skip_group_check=True)
            stage = stg.tile([128, 2, 256], F16, tag="stage")
            nc.vector.tensor_copy(stage[:], tout[:])
            nc.sync.dma_start(d_out[s, i, 0:128, :], stage[:, 0, :])
            nc.sync.dma_start(d_out[s, i, 128:130, :], stage[0:2, 1, :])

        # ---- emission: software-pipelined; iteration s runs step s's ctx
        # chain/agains/lstm2s and step s+1's vmaps (their sigmoids fill the
        # step-boundary latency) ----
        def vmaps(s, xa):
            L("vmap0"); g0, i0 = lstm1_mm(0, xa[0], f"v0_{s}")
            L("vmap1"); g1, i1 = lstm1_mm(1, xa[1], f"v1_{s}")
            L("vmap0t"); cell_tail(g0, i0, "h1_0", "h8first")
            L("vmap2"); g2, i2 = lstm1_mm(2, xa[2], f"v2_{s}")
            L("vmap1t"); cell_tail(g1, i1, "h1_1", "h8first")
            L("vmap2t"); cell_tail(g2, i2, "h1_2", "h8first")
            return ([st_h16[f"h1_{i}"] for i in range(3)],
                    [st_h8[f"h1_{i}"] for i in range(3)])

        pending = None  # deferred (lstm2_2 tail, out2) from prev step
        h1v16 = h1v8 = None
        for s in range(S):
            xb_cur = []
            for i in range(3):
                t = npool.tile([128, 8, 256], F16, tag=f"xb_{i}")
                nc.sync.dma_start(t[:], d_xb[s, i])
                xb_cur.append(t)
            if s == 0:
                h1v16, h1v8 = vmaps(0, xa0)
            crhs = dict(h1_0=h1v8[0], h1_1=h1v8[1], h1_2=h1v8[2],
                        hc=st_h8["hc"])

            L("ctx0"); cg, ci, cfin = ctx_mm(0, crhs, f"c0_{s}", defer=1)
            if pending is not None:
                pending()  # prev-step lstm2_2 tail + out2 fill the ctx0 wait
                pending = None
            with tc.high_priority(offset=PRIO):
                L("ctx0"); cfin()
            L("again0"); ag0, ai0 = lstm1_mm(0, xb_cur[0], f"a0_{s}")
            with tc.high_priority(offset=PRIO):
                L("ctx0t"); cell_tail(cg, ci, "hc", "h8only")
                L("again0t"); cell_tail(ag0, ai0, "h1_0", "h8only")
            L("again2"); ag2, ai2 = lstm1_mm(2, xb_cur[2], f"a2_{s}")
            L("lstm2_0"); lg, li, lfin = lstm2_mm(0, f"l0_{s}", defer=1)
            crhs = dict(crhs, h1_0=st_h8["h1_0"], hc=st_h8["hc"])
            L("ctx1"); cg, ci, cfin = ctx_mm(1, crhs, f"c1_{s}", defer=2)
            L("lstm2_0"); lfin()
            with tc.high_priority(offset=PRIO):
                L("ctx1"); cfin()
            if use_pair1:
                ppre, pfin = pair_tail("h1_2", "h8only", "h2_0", "h16pool")
                L("again2t"); ppre(0, ag2, ai2)
                L("lstm2_0t"); ppre(1, lg, li); pfin()
            else:
                L("again2t"); cell_tail(ag2, ai2, "h1_2", "h8only")
                L("lstm2_0t"); cell_tail(lg, li, "h2_0", "h16pool",
                                         rat=rat_l >= 1)
            with tc.high_priority(offset=PRIO):
                L("ctx1t"); cell_tail(cg, ci, "hc", "h8only")
            L("again1"); ag1, ai1 = lstm1_mm(1, xb_cur[1], f"a1_{s}")
            L("out0"); out_proj(0, h1v16[0], s)
            with tc.high_priority(offset=PRIO):
                L("again1t"); cell_tail(ag1, ai1, "h1_1", "h8only")
            L("lstm2_1"); lg, li, lfin = lstm2_mm(1, f"l1_{s}", defer=1)
            crhs = dict(crhs, h1_1=st_h8["h1_1"], hc=st_h8["hc"])
            nxt16 = nxt8 = None
            if s + 1 < S:
                # next step's vmap0/1 ahead of ctx2 in the PSUM ring so
                # their sigmoids can fill the step-boundary latency
                L("vmap0"); g0, i0 = lstm1_mm(0, xb_cur[0], f"v0_{s+1}")
            L("lstm2_1"); lfin()
            if s + 1 < S:
                L("vmap1"); g1, i1 = lstm1_mm(1, xb_cur[1], f"v1_{s+1}")
            L("ctx2"); cg, ci, cfin = ctx_mm(2, crhs, f"c2_{s}", defer=2)
            if s + 1 < S and use_pair2:
                qpre, qfin = pair_tail("h2_1", "h16pool", "h1_0", "h8first")
                L("lstm2_1t"); qpre(0, lg, li)
            else:
                L("lstm2_1t"); cell_tail(lg, li, "h2_1", "h16pool",
                                         rat=rat_l >= 2)
            with tc.high_priority(offset=PRIO):
                L("ctx2"); cfin()
            if s + 1 < S:
                if use_pair2:
                    L("vmap0t"); qpre(1, g0, i0); qfin()
                else:
                    with tc.high_priority(offset=PRIO):
                        L("vmap0t"); cell_tail(g0, i0, "h1_0", "h8first")
            L("out1"); out_proj(1, h1v16[1], s)
            with tc.high_priority(offset=PRIO):
                L("ctx2t"); cell_tail(cg, ci, "hc", "h8only")
            L("lstm2_2"); lg, li, _ = lstm2_mm(2, f"l2_{s}", defer=0)
            if s + 1 < S:
                with tc.high_priority(offset=PRIO):
                    L("vmap1t"); cell_tail(g1, i1, "h1_1", "h8first")
                    L("vmap2"); g2, i2 = lstm1_mm(2, xb_cur[2], f"v2_{s+1}")
                    L("vmap2t"); cell_tail(g2, i2, "h1_2", "h8first")
                nxt16 = [st_h16[f"h1_{i}"] for i in range(3)]
                nxt8 = [st_h8[f"h1_{i}"] for i in range(3)]

            def mk_pending(lg_, li_, h1v2_, s_):
                def f():
                    L("lstm2_2t"); cell_tail(lg_, li_, "h2_2", "h16pool")
                    L("out2"); out_proj(2, h1v2_, s_)
                return f
            pending = mk_pending(lg, li, h1v16[2], s)
            if s + 1 < S:
                h1v16, h1v8 = nxt16, nxt8
        pending()

    nc._state.pop_inst_callback()
    nc.compile()
    return nc


def kernel(c, target, length, W_hid, b_hid, W1_ih, W1_hh, b1_ih, b1_hh,
           Wc_ih, Wc_hh, bc_ih, bc_hh, emb, Wout, bout):
    global last_result
    c = np.asarray(c, np.float32)
    tgt = np.asarray(target).astype(np.int64)
    W_hid = np.asarray(W_hid, np.float32)
    b_hid = np.asarray(b_hid, np.float32)
    W1_ih = np.asarray(W1_ih, np.float32)[:, PERM4H]
    W1_hh = np.asarray(W1_hh, np.float32)[:, PERM4H]
    b1 = (np.asarray(b1_ih, np.float32) + np.asarray(b1_hh, np.float32))[:, PERM4H]
    Wc_ih = np.asarray(Wc_ih, np.float32)[PERM4H]
    Wc_hh = np.asarray(Wc_hh, np.float32)[PERM4H]
    bc = (np.asarray(bc_ih, np.float32) + np.asarray(bc_hh, np.float32))[PERM4H]
    emb = np.asarray(emb, np.float32)
    Wout = np.asarray(Wout, np.float32)
    bout = np.asarray(bout, np.float32)
    L = int(length)
    assert L == NB * S and c.shape == (B, NB + 1, Dd)

    use_ctx_bias = bool(np.any(bc != 0.0))

    # ---- replicated weight prep ----
    w1h8 = np.stack([_w8(WS * W1_hh[i]) for i in range(3)])
    w1n8 = np.stack([_w8(WS * W1_ih[i][:, :Dd]) for i in range(3)])
    wc8 = _w8(WS * np.concatenate([Wc_ih, Wc_hh], axis=1))
    if CFG["hh_resid"]:
        # residual shares the x16 psum scale: q(16W - q(16W))
        w1hr = np.stack([
            _w8(WS * W1_hh[i] -
                (WS * W1_hh[i]).astype(NPF8).astype(np.float32))
            for i in range(3)])
    # wo[p, k, v] = Wout[v, k*128+p]
    wo16 = np.stack([np.ascontiguousarray(
        Wout[i].T.reshape(2, 128, Vv).transpose(1, 0, 2)).astype(np.float16)
        for i in range(3)])
    bcb = _x16(np.broadcast_to(bc[None, :], (R, 4 * Hh)))

    # full-batch fp32 precomputes
    h_init_full = np.tanh(np.einsum('bnd,hd->bnh', c[:, :NB], W_hid[:Hh]) +
                          b_hid[:Hh])
    NEt = np.stack([emb[i] @ W1_ih[i][:, :Dd].T for i in range(3)])
    in_maps = []
    for r in range(NCORES):
        cs = c[r * BL:(r + 1) * BL]
        CT = cs[:, 1:NB + 1].transpose(1, 0, 2).reshape(R, Dd)
        HI = h_init_full[r * BL:(r + 1) * BL].transpose(1, 0, 2).reshape(R, Hh)
        xc1f = [CT @ W1_ih[i][:, Dd:].T + b1[i] for i in range(3)]
        xc1 = np.stack([_x16(x) for x in xc1f])
        hinit16 = _fold(HI).astype(np.float16)
        hinit8 = hinit16.astype(NPF8)
        tg = tgt[:, r * BL:(r + 1) * BL]
        tokA0 = np.empty((3, R), np.int64)
        for i in range(3):
            tokA0[i] = np.concatenate(
                [np.zeros(BL, np.int64)] +
                [tg[i, :, bar * S - 1] for bar in range(1, NB)])
        xa0 = np.stack([_x16(NEt[i][tokA0[i]] + xc1f[i]) for i in range(3)])
        tr = tg.reshape(3, BL, NB, S)
        xbarr = np.empty((S, 3, 128, 8, 256), np.float16)
        for s in range(S):
            for i in range(3):
                toks = tr[i, :, :, s].T.reshape(R)
                xbarr[s, i] = _x16(NEt[i][toks] + xc1f[i])
        m = dict(w1h=w1h8, w1n=w1n8, wc=wc8, wo=wo16, xc1=xc1,
                 hinit8=hinit8, hinit16=hinit16, xa0=xa0, xb=xbarr)
        if CFG["hh_resid"]:
            m["w1hr"] = w1hr
        if use_ctx_bias:
            m["bcb"] = bcb
        in_maps.append(m)

    key = (use_ctx_bias, tuple(sorted(CFG.items())))
    if key not in _prog_cache:
        _prog_cache[key] = _build_program(key)
    nc = _prog_cache[key]

    last_result = run_bass_kernel_spmd(nc, in_maps, core_ids=list(range(NCORES)))

    out_full = np.empty((3, B, L, Vv), np.float32)
    for r in range(NCORES):
        A = np.asarray(last_result.results[r]["out"], np.float32)  # [S,3,130,R]
        A = A.reshape(S, 3, Vv, NB, BL).transpose(1, 4, 3, 0, 2)
        out_full[:, r * BL:(r + 1) * BL] = A.reshape(3, BL, L, Vv)
    if np.any(bout):
        out_full += bout[:, None, None, :]
    return out_full
